# revision 4
# baseline (speedup 1.0000x reference)
"""Dice-coefficient-mean kernel v3 for Trainium2 (8 NeuronCores, SPMD).

Sharding: data-parallel over batch - core b processes batch b
(128^3 = 2,097,152 elements per tensor, laid out [128, 16384]).

Needed per core, per label l (42 numbers):
  inter[l] = #(s1==l & s2==l), c1[l] = #(s1==l), c2[l] = #(s2==l)

Engine model (HW-measured in earlier sessions):
  DVE tensor_scalar mask fp16 4x: (FD/4+150)/0.96GHz per instr
  DVE TT/STT fp16 2x:            (FD/2+150)/0.96
  ScalarE ACTIVATE 1x:           (FD+352)/1.2 (dtype-independent)
  PE ones-matmul N=512 with 4-way column tiling: ~59ns
  GPSIMD elementwise shares an exclusive SBUF port with DVE 2-port
  (4x) ops - useless here; cast-DMA (f32->fp16) is SWDGE.

v3 changes vs v2 (191us):
  - pair = 16*s1 + s2 fused into ONE scalar_tensor_tensor (2x) -
    drops the separate s1x mult; c1 thresholds run on raw s1 at t+0.5.
  - 3 DMA chunks per tensor (2048/6144/8192 cols), order
    s1a,s2a | s1b,s2b,s1c,s2c with the late 4 serialized by tiny
    marker DMAs on the (idle) sync queue: RAW on the previous chunk,
    WAW with the next -> SWDGE desc-gen waits on a semaphore (no
    gpsimd SBUF-port use) instead of port-blocking stall ops.
  - ScalarE starts at ~13us (first s2 chunk is 1MB) instead of 43us.
  - per-(stat, chunk-piece) engine assignment via a manifest; host
    decode sums pieces. ACT takes c2 T0..7 everywhere + small fills;
    DVE takes the rest. Final PSUM bank reduces on ScalarE.

Decode in float64: c1/c2 from 12 cums + m1 + N (tails c12,c13 via
2x2 integer Vandermonde); inter directly from 14 eq counts.
All counts are exact integers in fp32.
"""

import numpy as np

NUM_LABELS = 14
EPS = float(np.finfo(float).eps)
B = 8
P = 128
FREE = 16384
MM_N = 512
NBANK = 8

# chunk column ranges (a, b, c)
CHUNKS = [(0, 2048), (2048, 8192), (8192, 16384)]

USE_STT = True  # pair via one scalar_tensor_tensor (2x); else mult+add

_CACHE = {}


def _slot(j):
    bank = j % NBANK
    pos = 32 * ((bank + j // NBANK) % 4)
    return bank, pos


# ---------------------------------------------------------------------------
# stat table
#   kind: ("c1", t) is_le t+0.5 on s1 | ("c2", t) on s2 | ("eq", l) on pair
# assignment per piece index 0/1/2 -> "dve" or "act"
# ---------------------------------------------------------------------------


def _stats():
    stats = []
    for t in range(12):
        eng = ["act"] * 3 if t <= 7 else (
            ["act", "act", "dve"] if t == 8 else ["dve"] * 3
        )
        stats.append(("c2", t, eng))
    for t in range(12):
        if t <= 9:
            eng = ["dve"] * 3
        else:
            eng = ["act", "dve", "dve"]  # T10,11: a-piece fills ACT idle
        stats.append(("c1", t, eng))
    for l in range(NUM_LABELS):
        stats.append(("eq", l, ["dve"] * 3))
    return stats


def _build():
    from concourse import bacc, mybir, tile

    op = mybir.AluOpType
    stats = _stats()

    # DVE chain slots: stats with any dve piece, + 2 moment chains
    dve_stats = [s for s in stats if "dve" in s[2]]
    n_chain = len(dve_stats) + 2
    assert n_chain <= 32, n_chain

    nc = bacc.Bacc("TRN2", target_bir_lowering=False)
    s1 = nc.dram_tensor("s1", [P, FREE], mybir.dt.float32, kind="ExternalInput")
    s2 = nc.dram_tensor("s2", [P, FREE], mybir.dt.float32, kind="ExternalInput")
    out_p = nc.dram_tensor(
        "stats_pe", [P, NBANK], mybir.dt.float32, kind="ExternalOutput"
    )
    N_ACTS = sum(1 for s in stats for e in s[2] if e == "act") + 1  # + warm
    out_a = nc.dram_tensor(
        "stats_act", [P, N_ACTS], mybir.dt.float32, kind="ExternalOutput"
    )

    manifest = {"psum": {}, "acta": {}, "n_acts": N_ACTS}

    with tile.TileContext(nc) as tc:
        with (
            tc.tile_pool(name="data", bufs=1) as dpool,
            tc.tile_pool(name="mask", bufs=4) as maskp,
            tc.tile_pool(name="aux", bufs=1) as aux,
            tc.tile_pool(name="psum", bufs=1, space="PSUM") as psum,
        ):
            s1h = dpool.tile([P, FREE], mybir.dt.float16, name="s1h")
            s2h = dpool.tile([P, FREE], mybir.dt.float16, name="s2h")
            pair = dpool.tile([P, FREE], mybir.dt.float16, name="pair")
            stats_p = aux.tile([P, NBANK], mybir.dt.float32)
            stats_a = aux.tile([P, N_ACTS], mybir.dt.float32)
            junk = aux.tile([P, FREE // 2], mybir.dt.float8e4)
            junk32 = aux.tile([P, MM_N], mybir.dt.float32)
            ones = aux.tile([P, 1], mybir.dt.float16)
            biases = aux.tile([P, 15], mybir.dt.float32)
            nc.vector.memset(ones[:], 1.0)
            nc.vector.memset(stats_p[:], 0.0)
            for i in range(14):
                nc.vector.memset(biases[:, i:i + 1], -(i + 0.5))
            nc.vector.memset(biases[:, 14:15], 0.0)
            accs = [
                psum.tile([P, MM_N], mybir.dt.float32, tag=f"acc{g}",
                          name=f"acc{g}")
                for g in range(NBANK)
            ]

            warm = aux.tile([P, 1], mybir.dt.float16)
            nc.scalar.activation(
                out=warm[:], in_=ones[:],
                func=mybir.ActivationFunctionType.Sign,
                bias=biases[:, 0:1], scale=1.0,
            )

            # ---- DMA: chunked cast loads --------------------------------
            # first two immediately (share bandwidth, both ~1MB);
            # rest serialized via sync-queue marker DMAs.
            def big_dma(dst, src, lo, hi):
                nc.gpsimd.dma_start(out=dst[:, lo:hi], in_=src[:, lo:hi])

            big_dma(s1h, s1, *CHUNKS[0])
            big_dma(s2h, s2, *CHUNKS[0])
            # chain: s1b after s2a, s2b after s1b, s1c after s2b, s2c after s1c
            chain = [
                (s2h, CHUNKS[0], s1h, s1, CHUNKS[1]),
                (s1h, CHUNKS[1], s2h, s2, CHUNKS[1]),
                (s2h, CHUNKS[1], s1h, s1, CHUNKS[2]),
                (s1h, CHUNKS[2], s2h, s2, CHUNKS[2]),
            ]
            for prev_t, prev_rng, dst, src, rng in chain:
                # marker: read 1 col of the completed chunk, write 1 col of
                # the next chunk's range (WAW gates the next SWDGE gen)
                nc.sync.dma_start(
                    out=dst[:, rng[0]:rng[0] + 1],
                    in_=prev_t[:, prev_rng[0]:prev_rng[0] + 1],
                )
                big_dma(dst, src, *rng)

            # ---- PE pump machinery --------------------------------------
            streams = []  # [acc, pos, rhs_tile, lo_col, next_k, nmm, start, stop]

            def pump(tokens, min_live=2):
                while tokens > 0 and len(streams) >= min_live:
                    for st in list(streams):
                        if tokens <= 0:
                            break
                        acc, pos, rhs, lo, k, nmm, first, last = st
                        nc.tensor.matmul(
                            acc[pos:pos + 1, :],
                            ones[:],
                            rhs[:, lo + k * MM_N:lo + (k + 1) * MM_N],
                            start=(first and k == 0),
                            stop=(last and k == nmm - 1),
                            tile_position=(0, pos),
                        )
                        st[4] += 1
                        tokens -= 1
                        if st[4] >= nmm:
                            streams.remove(st)

            # chain slot allocation in dve emission order
            chain_slot = {}
            slot_i = 0
            for s in dve_stats:
                chain_slot[(s[0], s[1])] = _slot(slot_i)
                slot_i += 1
            chain_slot[("m1", "s1")] = _slot(slot_i)
            chain_slot[("m1", "s2")] = _slot(slot_i + 1)
            for key, (bank, pos) in chain_slot.items():
                manifest["psum"].setdefault(key, (bank, pos))

            # per-stat piece lists on dve (to place start/stop flags)
            dve_pieces = {}
            for kind, v, eng in stats:
                dve_pieces[(kind, v)] = [i for i, e in enumerate(eng)
                                         if e == "dve"]
            dve_pieces[("m1", "s1")] = [0, 1, 2]
            dve_pieces[("m1", "s2")] = [0, 1, 2]

            act_col = [0]

            def act_stat(kind, v, piece):
                lo, hi = CHUNKS[piece]
                src = s1h if kind == "c1" else s2h
                col = act_col[0]
                act_col[0] += 1
                manifest["acta"].setdefault((kind, v), []).append(
                    (col, piece, hi - lo))
                nc.scalar.activation(
                    out=junk[:, 0:hi - lo], in_=src[:, lo:hi],
                    func=mybir.ActivationFunctionType.Sign,
                    bias=biases[:, v:v + 1], scale=1.0,
                    accum_out=stats_a[:, col:col + 1],
                )

            def dve_stat(kind, v, piece):
                lo, hi = CHUNKS[piece]
                bank, pos = chain_slot[(kind, v)]
                pieces = dve_pieces[(kind, v)]
                first = piece == pieces[0]
                last = piece == pieces[-1]
                nmm = (hi - lo) // MM_N
                if kind == "m1":
                    rhs = s1h if v == "s1" else s2h
                else:
                    mask = maskp.tile([P, FREE // 2], mybir.dt.float16,
                                      tag="mask")
                    if kind == "eq":
                        nc.vector.tensor_scalar(
                            out=mask[:, 0:hi - lo], in0=pair[:, lo:hi],
                            scalar1=17.0 * v, scalar2=None, op0=op.is_equal,
                        )
                    else:
                        src = s1h if kind == "c1" else s2h
                        nc.vector.tensor_scalar(
                            out=mask[:, 0:hi - lo], in0=src[:, lo:hi],
                            scalar1=v + 0.5, scalar2=None, op0=op.is_le,
                        )
                    rhs, lo = mask, 0
                streams.append([accs[bank], pos, rhs, lo, 0, nmm, first, last])
                pump(nmm)

            def pair_prep(piece):
                lo, hi = CHUNKS[piece]
                if USE_STT:
                    nc.vector.scalar_tensor_tensor(
                        out=pair[:, lo:hi], in0=s1h[:, lo:hi], scalar=16.0,
                        in1=s2h[:, lo:hi], op0=op.mult, op1=op.add,
                    )
                else:
                    nc.vector.tensor_scalar(
                        out=pair[:, lo:hi], in0=s1h[:, lo:hi], scalar1=16.0,
                        scalar2=None, op0=op.mult,
                    )
                    nc.vector.tensor_tensor(
                        out=pair[:, lo:hi], in0=pair[:, lo:hi],
                        in1=s2h[:, lo:hi], op=op.add,
                    )

            # ---- emission schedule --------------------------------------
            # DVE queue per piece-phase: c1 thr, c2 thr(dve), pair, eq, m1
            # ACT queue: its pieces in data-arrival order
            for piece in range(3):
                # ACT work for this phase (emitted first; independent queue)
                for kind, v, eng in stats:
                    if eng[piece] == "act":
                        act_stat(kind, v, piece)
                # DVE work
                for kind, v, eng in stats:
                    if kind == "c1" and eng[piece] == "dve":
                        dve_stat(kind, v, piece)
                dve_stat("m1", "s1", piece)
                for kind, v, eng in stats:
                    if kind == "c2" and eng[piece] == "dve":
                        dve_stat(kind, v, piece)
                dve_stat("m1", "s2", piece)
                pair_prep(piece)
                for kind, v, eng in stats:
                    if kind == "eq" and eng[piece] == "dve":
                        dve_stat(kind, v, piece)
                while streams:
                    pump(10 ** 9, min_live=1)

            # ---- tail: bank reduces on ScalarE, then DMA out ------------
            for bank in range(NBANK):
                nc.scalar.activation(
                    out=junk32[:], in_=accs[bank][:],
                    func=mybir.ActivationFunctionType.Identity,
                    bias=biases[:, 14:15], scale=1.0,
                    accum_out=stats_p[:, bank:bank + 1],
                )
            nc.sync.dma_start(out=out_p[:], in_=stats_p[:])
            nc.sync.dma_start(out=out_a[:], in_=stats_a[:])
    nc.compile()
    return nc, stats, manifest


def _get_built():
    if "k" not in _CACHE:
        _CACHE["k"] = _build()
    return _CACHE["k"]


LAST_EXEC_NS = None
LAST_RESULTS = None


def _decode(results, stats, manifest):
    n_total = float(P * FREE)
    dice = np.zeros((B, NUM_LABELS), dtype=np.float64)
    for b in range(B):
        sp = np.asarray(results[b]["stats_pe"], dtype=np.float64)  # [P, NBANK]
        sa = np.asarray(results[b]["stats_act"], dtype=np.float64)

        def stat_val(kind, v):
            tot = 0.0
            if (kind, v) in manifest["psum"]:
                has_dve = any(
                    e == "dve"
                    for k2, v2, e2 in stats if (k2, v2) == (kind, v)
                    for e in e2
                ) or kind == "m1"
                if has_dve:
                    bank, pos = manifest["psum"][(kind, v)]
                    tot += sp[pos, bank]
            for col, piece, fd in manifest["acta"].get((kind, v), ()):
                # Sign sum: accum = (#gt - #le); count_le = (fd - accum)/2
                tot += (fd - sa[:, col]).sum() / 2.0
            return tot

        inter = np.array([stat_val("eq", l) for l in range(NUM_LABELS)])
        f1 = np.array([stat_val("c1", t) for t in range(12)])
        f2 = np.array([stat_val("c2", t) for t in range(12)])
        mom1 = stat_val("m1", "s1")
        mom2 = stat_val("m1", "s2")
        c1 = np.zeros(NUM_LABELS)
        c2 = np.zeros(NUM_LABELS)
        c1[0] = f1[0]
        c2[0] = f2[0]
        for t in range(1, 12):
            c1[t] = f1[t] - f1[t - 1]
            c2[t] = f2[t] - f2[t - 1]
        r1 = n_total - f1[11]
        m1r = mom1 - sum(v * c1[v] for v in range(12))
        c1[13] = m1r - 12.0 * r1
        c1[12] = r1 - c1[13]
        r2 = n_total - f2[11]
        m2r = mom2 - sum(v * c2[v] for v in range(12))
        c2[13] = m2r - 12.0 * r2
        c2[12] = r2 - c2[13]
        dice[b] = 2.0 * inter / (c1 + c2 + EPS)
    resv = dice.reshape(-1)
    total = resv.sum()
    nz = float((resv > 0).sum())
    mean = total / nz if nz > 0 else 0.0
    return np.float32(mean)


def _run(segment1, segment2, trace=False):
    global LAST_EXEC_NS, LAST_RESULTS
    from concourse.bass_utils import run_bass_kernel_spmd

    nc, stats, manifest = _get_built()
    seg1 = np.ascontiguousarray(np.asarray(segment1, dtype=np.float32)).reshape(
        B, P, FREE
    )
    seg2 = np.ascontiguousarray(np.asarray(segment2, dtype=np.float32)).reshape(
        B, P, FREE
    )
    in_maps = [{"s1": seg1[b], "s2": seg2[b]} for b in range(B)]
    res = run_bass_kernel_spmd(nc, in_maps, core_ids=list(range(B)), trace=trace)
    LAST_EXEC_NS = res.exec_time_ns
    LAST_RESULTS = res
    return _decode(res.results, stats, manifest)


def kernel(segment1, segment2):
    return _run(segment1, segment2, trace=False)


def benchmark(segment1, segment2):
    _run(segment1, segment2, trace=True)
    return LAST_EXEC_NS


# revision 17
# speedup vs baseline: 1.0452x; 1.0452x over previous
"""Dice-coefficient-mean kernel v3 for Trainium2 (8 NeuronCores, SPMD).

Sharding: data-parallel over batch - core b processes batch b
(128^3 = 2,097,152 elements per tensor, laid out [128, 16384]).

Needed per core, per label l (42 numbers):
  inter[l] = #(s1==l & s2==l), c1[l] = #(s1==l), c2[l] = #(s2==l)

Engine model (HW-measured in earlier sessions):
  DVE tensor_scalar mask fp16 4x: (FD/4+150)/0.96GHz per instr
  DVE TT/STT fp16 2x:            (FD/2+150)/0.96
  ScalarE ACTIVATE 1x:           (FD+352)/1.2 (dtype-independent)
  PE ones-matmul N=512 with 4-way column tiling: ~59ns
  GPSIMD elementwise shares an exclusive SBUF port with DVE 2-port
  (4x) ops - useless here; cast-DMA (f32->fp16) is SWDGE.

v3 changes vs v2 (191us):
  - pair = 16*s1 + s2 fused into ONE scalar_tensor_tensor (2x) -
    drops the separate s1x mult; c1 thresholds run on raw s1 at t+0.5.
  - 3 DMA chunks per tensor (2048/6144/8192 cols), order
    s1a,s2a | s1b,s2b,s1c,s2c with the late 4 serialized by tiny
    marker DMAs on the (idle) sync queue: RAW on the previous chunk,
    WAW with the next -> SWDGE desc-gen waits on a semaphore (no
    gpsimd SBUF-port use) instead of port-blocking stall ops.
  - ScalarE starts at ~13us (first s2 chunk is 1MB) instead of 43us.
  - per-(stat, chunk-piece) engine assignment via a manifest; host
    decode sums pieces. ACT takes c2 T0..7 everywhere + small fills;
    DVE takes the rest. Final PSUM bank reduces on ScalarE.

Decode in float64: c1/c2 from 12 cums + m1 + N (tails c12,c13 via
2x2 integer Vandermonde); inter directly from 14 eq counts.
All counts are exact integers in fp32.
"""

import numpy as np

NUM_LABELS = 14
EPS = float(np.finfo(float).eps)
B = 8
P = 128
FREE = 16384
MM_N = 512
NBANK = 8

# DMA chunk ranges overlap by 512 cols: the WAW hazard on the overlap
# serializes each tensor's chunk chain (s1a->s1b->s1c, s2a->s2b->s2c)
# with ZERO extra instructions; the two chains share DMA bandwidth
# ~50/50. Compute phases read only the non-overlapped 512-aligned
# ranges, so phase-A work never waits on chunk b/c.
DMA_CHUNKS = [(0, 3072), (2560, 8192), (7680, 16384)]
CHUNKS = [(0, 2560), (2560, 7680), (7680, 16384)]  # compute phases

USE_STT = False  # HW-measured: scalar_tensor_tensor runs 1x; mult+add 4x/2x wins

_CACHE = {}


def _slot(j):
    bank = j % NBANK
    pos = 32 * ((bank + j // NBANK) % 4)
    return bank, pos


# ---------------------------------------------------------------------------
# stat table
#   kind: ("c1", t) is_le t+0.5 on s1 | ("c2", t) on s2 | ("eq", l) on pair
# assignment per piece index 0/1/2 -> "dve" or "act"
# ---------------------------------------------------------------------------


def _stats():
    stats = []
    for t in range(12):
        if t <= 8:
            eng = ["act"] * 3
        elif t == 9:
            eng = ["dve", "dve", "act"]  # c-piece balances ACT vs DVE end
        else:
            eng = ["dve"] * 3
        stats.append(("c2", t, eng))
    for t in range(12):
        if t <= 9:
            eng = ["dve"] * 3
        else:
            eng = ["act", "dve", "dve"]  # T10,11: a-piece fills ACT idle
        stats.append(("c1", t, eng))
    for l in range(NUM_LABELS):
        stats.append(("eq", l, ["dve"] * 3))
    return stats


def _completion_order_slots(stats):
    """Chain slot per dve stat, assigned in estimated completion order
    (phase-c emission order), quad-grouped so banks 0-3 close before the
    eq chains and their reduces overlap the eq-c masks.

    Within each group of 16, consecutive chains get distinct banks AND
    distinct tile positions (PE column tiles run concurrently only then).
    """
    order = []
    for kind, v, eng in stats:
        if kind == "c1" and "dve" in eng:
            order.append((kind, v))
    order.append(("m1", "s1"))
    for kind, v, eng in stats:
        if kind == "c2" and "dve" in eng:
            order.append((kind, v))
    order.append(("m1", "s2"))
    for kind, v, eng in stats:
        if kind == "eq":
            order.append((kind, v))
    slots = {}
    for j, key in enumerate(order):
        base = 0 if j < 16 else 4
        jj = j % 16
        bank = base + jj % 4
        pos = 32 * ((jj % 4 + jj // 4) % 4)
        slots[key] = (bank, pos)
    return slots


def _build():
    from concourse import bacc, mybir, tile

    op = mybir.AluOpType
    stats = _stats()

    # DVE chain slots: stats with any dve piece, + 2 moment chains
    dve_stats = [s for s in stats if "dve" in s[2]]
    n_chain = len(dve_stats) + 2
    assert n_chain <= 32, n_chain

    nc = bacc.Bacc("TRN2", target_bir_lowering=False)
    s1 = nc.dram_tensor("s1", [P, FREE], mybir.dt.float32, kind="ExternalInput")
    s2 = nc.dram_tensor("s2", [P, FREE], mybir.dt.float32, kind="ExternalInput")
    out_p = nc.dram_tensor(
        "stats_pe", [P, NBANK], mybir.dt.float32, kind="ExternalOutput"
    )
    N_ACTS = sum(1 for s in stats for e in s[2] if e == "act") + 1  # + warm
    out_a = nc.dram_tensor(
        "stats_act", [P, N_ACTS], mybir.dt.float32, kind="ExternalOutput"
    )

    manifest = {"psum": {}, "acta": {}, "n_acts": N_ACTS}

    with tile.TileContext(nc) as tc:
        with (
            tc.tile_pool(name="data", bufs=1) as dpool,
            tc.tile_pool(name="mask", bufs=4) as maskp,
            tc.tile_pool(name="aux", bufs=1) as aux,
            tc.tile_pool(name="psum", bufs=1, space="PSUM") as psum,
        ):
            s1h = dpool.tile([P, FREE], mybir.dt.float16, name="s1h")
            s2h = dpool.tile([P, FREE], mybir.dt.float16, name="s2h")
            pair = dpool.tile([P, FREE], mybir.dt.float16, name="pair")
            stats_p = aux.tile([P, NBANK], mybir.dt.float32)
            stats_a = aux.tile([P, N_ACTS], mybir.dt.float32)
            junk = aux.tile([P, 8704], mybir.dt.float8e4)
            junk32 = aux.tile([P, MM_N], mybir.dt.float32)
            ones = aux.tile([P, 1], mybir.dt.float16)
            biases = aux.tile([P, 15], mybir.dt.float32)
            nc.vector.memset(ones[:], 1.0)
            nc.vector.memset(stats_p[:], 0.0)
            for i in range(14):
                nc.vector.memset(biases[:, i:i + 1], -(i + 0.5))
            nc.vector.memset(biases[:, 14:15], 0.0)
            accs = [
                psum.tile([P, MM_N], mybir.dt.float32, tag=f"acc{g}",
                          name=f"acc{g}")
                for g in range(NBANK)
            ]

            warm = aux.tile([P, 1], mybir.dt.float16)
            nc.scalar.activation(
                out=warm[:], in_=ones[:],
                func=mybir.ActivationFunctionType.Sign,
                bias=biases[:, 0:1], scale=1.0,
            )

            # ---- DMA: chunked cast loads --------------------------------
            # first two immediately (share bandwidth, both ~1MB);
            # rest serialized via sync-queue marker DMAs.
            def big_dma(dst, src, lo, hi):
                nc.gpsimd.dma_start(out=dst[:, lo:hi], in_=src[:, lo:hi])

            # Two concurrent per-tensor serial chains via overlap-column WAW
            for rng in DMA_CHUNKS:
                big_dma(s1h, s1, *rng)
                big_dma(s2h, s2, *rng)

            # ---- PE pump machinery --------------------------------------
            streams = []  # [acc, pos, rhs_tile, lo_col, next_k, nmm, start, stop]

            def pump(tokens, min_live=2):
                while tokens > 0 and len(streams) >= min_live:
                    for st in list(streams):
                        if tokens <= 0:
                            break
                        acc, pos, rhs, lo, k, nmm, first, last = st
                        nc.tensor.matmul(
                            acc[pos:pos + 1, :],
                            ones[:],
                            rhs[:, lo + k * MM_N:lo + (k + 1) * MM_N],
                            start=(first and k == 0),
                            stop=(last and k == nmm - 1),
                            tile_position=(0, pos),
                        )
                        st[4] += 1
                        tokens -= 1
                        if st[4] >= nmm:
                            streams.remove(st)

            # chain slots in completion order: banks 0-3 close before eq-c
            chain_slot = _completion_order_slots(stats)
            for key, (bank, pos) in chain_slot.items():
                manifest["psum"].setdefault(key, (bank, pos))

            # per-stat piece lists on dve (to place start/stop flags)
            dve_pieces = {}
            for kind, v, eng in stats:
                dve_pieces[(kind, v)] = [i for i, e in enumerate(eng)
                                         if e == "dve"]
            dve_pieces[("m1", "s1")] = [0, 1, 2]
            dve_pieces[("m1", "s2")] = [0, 1, 2]

            act_col = [0]

            def act_stat(kind, v, piece):
                lo, hi = CHUNKS[piece]
                src = s1h if kind == "c1" else s2h
                col = act_col[0]
                act_col[0] += 1
                manifest["acta"].setdefault((kind, v), []).append(
                    (col, piece, hi - lo))
                nc.scalar.activation(
                    out=junk[:, 0:hi - lo], in_=src[:, lo:hi],
                    func=mybir.ActivationFunctionType.Sign,
                    bias=biases[:, v:v + 1], scale=1.0,
                    accum_out=stats_a[:, col:col + 1],
                )

            def dve_stat(kind, v, piece):
                lo, hi = CHUNKS[piece]
                bank, pos = chain_slot[(kind, v)]
                pieces = dve_pieces[(kind, v)]
                first = piece == pieces[0]
                last = piece == pieces[-1]
                nmm = (hi - lo) // MM_N
                if kind == "m1":
                    rhs = s1h if v == "s1" else s2h
                else:
                    mask = maskp.tile([P, 8704], mybir.dt.float16,
                                      tag="mask")
                    if kind == "eq":
                        nc.vector.tensor_scalar(
                            out=mask[:, 0:hi - lo], in0=pair[:, lo:hi],
                            scalar1=17.0 * v, scalar2=None, op0=op.is_equal,
                        )
                    else:
                        src = s1h if kind == "c1" else s2h
                        nc.vector.tensor_scalar(
                            out=mask[:, 0:hi - lo], in0=src[:, lo:hi],
                            scalar1=v + 0.5, scalar2=None, op0=op.is_le,
                        )
                    rhs, lo = mask, 0
                streams.append([accs[bank], pos, rhs, lo, 0, nmm, first, last])
                pump(nmm)

            def pair_prep(piece):
                lo, hi = CHUNKS[piece]
                if USE_STT:
                    nc.vector.scalar_tensor_tensor(
                        out=pair[:, lo:hi], in0=s1h[:, lo:hi], scalar=16.0,
                        in1=s2h[:, lo:hi], op0=op.mult, op1=op.add,
                    )
                else:
                    nc.vector.tensor_scalar(
                        out=pair[:, lo:hi], in0=s1h[:, lo:hi], scalar1=16.0,
                        scalar2=None, op0=op.mult,
                    )
                    nc.vector.tensor_tensor(
                        out=pair[:, lo:hi], in0=pair[:, lo:hi],
                        in1=s2h[:, lo:hi], op=op.add,
                    )

            # ---- emission schedule --------------------------------------
            # DVE queue per piece-phase: c1 thr, c2 thr(dve), pair, eq, m1
            # ACT queue: its pieces in data-arrival order
            def bank_reduce(bank):
                nc.scalar.activation(
                    out=junk32[:], in_=accs[bank][:],
                    func=mybir.ActivationFunctionType.Identity,
                    bias=biases[:, 14:15], scale=1.0,
                    accum_out=stats_p[:, bank:bank + 1],
                )

            for piece in range(3):
                # ACT work for this phase (emitted first; independent queue)
                for kind, v, eng in stats:
                    if eng[piece] == "act":
                        act_stat(kind, v, piece)
                # DVE work
                for kind, v, eng in stats:
                    if kind == "c1" and eng[piece] == "dve":
                        dve_stat(kind, v, piece)
                dve_stat("m1", "s1", piece)
                for kind, v, eng in stats:
                    if kind == "c2" and eng[piece] == "dve":
                        dve_stat(kind, v, piece)
                dve_stat("m1", "s2", piece)
                if piece == 2:
                    # all non-eq chains (banks 0-3 + m1s2) fully emitted;
                    # drain their matmuls, then their reduces overlap the
                    # eq-c phase on the ACT queue
                    while streams:
                        pump(10 ** 9, min_live=1)
                    for bank in range(4):
                        bank_reduce(bank)
                pair_prep(piece)
                for kind, v, eng in stats:
                    if kind == "eq" and eng[piece] == "dve":
                        dve_stat(kind, v, piece)
                while streams:
                    pump(10 ** 9, min_live=1)

            # ---- tail: late bank reduces on ScalarE, then DMA out -------
            for bank in range(4, NBANK):
                bank_reduce(bank)
            nc.sync.dma_start(out=out_p[:], in_=stats_p[:])
            nc.sync.dma_start(out=out_a[:], in_=stats_a[:])
    nc.compile()
    return nc, stats, manifest


def _get_built():
    if "k" not in _CACHE:
        _CACHE["k"] = _build()
    return _CACHE["k"]


LAST_EXEC_NS = None
LAST_RESULTS = None


def _decode(results, stats, manifest):
    n_total = float(P * FREE)
    dice = np.zeros((B, NUM_LABELS), dtype=np.float64)
    for b in range(B):
        sp = np.asarray(results[b]["stats_pe"], dtype=np.float64)  # [P, NBANK]
        sa = np.asarray(results[b]["stats_act"], dtype=np.float64)

        def stat_val(kind, v):
            tot = 0.0
            if (kind, v) in manifest["psum"]:
                has_dve = any(
                    e == "dve"
                    for k2, v2, e2 in stats if (k2, v2) == (kind, v)
                    for e in e2
                ) or kind == "m1"
                if has_dve:
                    bank, pos = manifest["psum"][(kind, v)]
                    tot += sp[pos, bank]
            for col, piece, fd in manifest["acta"].get((kind, v), ()):
                # Sign sum: accum = (#gt - #le); count_le = (fd - accum)/2
                tot += (fd - sa[:, col]).sum() / 2.0
            return tot

        inter = np.array([stat_val("eq", l) for l in range(NUM_LABELS)])
        f1 = np.array([stat_val("c1", t) for t in range(12)])
        f2 = np.array([stat_val("c2", t) for t in range(12)])
        mom1 = stat_val("m1", "s1")
        mom2 = stat_val("m1", "s2")
        c1 = np.zeros(NUM_LABELS)
        c2 = np.zeros(NUM_LABELS)
        c1[0] = f1[0]
        c2[0] = f2[0]
        for t in range(1, 12):
            c1[t] = f1[t] - f1[t - 1]
            c2[t] = f2[t] - f2[t - 1]
        r1 = n_total - f1[11]
        m1r = mom1 - sum(v * c1[v] for v in range(12))
        c1[13] = m1r - 12.0 * r1
        c1[12] = r1 - c1[13]
        r2 = n_total - f2[11]
        m2r = mom2 - sum(v * c2[v] for v in range(12))
        c2[13] = m2r - 12.0 * r2
        c2[12] = r2 - c2[13]
        dice[b] = 2.0 * inter / (c1 + c2 + EPS)
    resv = dice.reshape(-1)
    total = resv.sum()
    nz = float((resv > 0).sum())
    mean = total / nz if nz > 0 else 0.0
    return np.float32(mean)


def _run(segment1, segment2, trace=False):
    global LAST_EXEC_NS, LAST_RESULTS
    from concourse.bass_utils import run_bass_kernel_spmd

    nc, stats, manifest = _get_built()
    seg1 = np.ascontiguousarray(np.asarray(segment1, dtype=np.float32)).reshape(
        B, P, FREE
    )
    seg2 = np.ascontiguousarray(np.asarray(segment2, dtype=np.float32)).reshape(
        B, P, FREE
    )
    in_maps = [{"s1": seg1[b], "s2": seg2[b]} for b in range(B)]
    res = run_bass_kernel_spmd(nc, in_maps, core_ids=list(range(B)), trace=trace)
    LAST_EXEC_NS = res.exec_time_ns
    LAST_RESULTS = res
    return _decode(res.results, stats, manifest)


def kernel(segment1, segment2):
    return _run(segment1, segment2, trace=False)


def benchmark(segment1, segment2):
    _run(segment1, segment2, trace=True)
    return LAST_EXEC_NS
